# revision 11
# baseline (speedup 1.0000x reference)
"""BiCutLoss Trainium2 kernel (8-core data parallel over batch).

Reference semantics (B=16384, L=1024):
    temp[b,j]  = argmax(output[b,j,:])          # 1 iff out1 > out0 (ties -> 0)
    idx[b]     = L if row all-ones else index of last zero
    mask[b,j]  = j <= idx[b]
    r1[b,j]    = -1/log2(j+2)  if labels==1 else (j+1)/alpha
    loss       = sum(output[...,1] * mask * r1) / B

Restructuring: masked_sum = full_sum - tail_sum; the tail (j > idx) is
confined to the last W columns whenever each row has a zero decision in
its last W positions (P(violation) = 2^-W per random row; flags catch it
and the host falls back to an exact evaluation).

v2 design (memory-regime):
  - All value data f16 (halves HBM traffic; DVE 2x mode; PE 1 cyc/row).
  - Labels travel as u8 (2MB/core) and are converted u8->f16 on ScalarE
    (otherwise idle), so ql = out1*lab runs at DVE 2x.
  - Column sums via PE matmuls with one-hot [128,4] stationaries into a
    single PSUM tile [4,512] per chain: row0 = colsum(out1) j<512,
    row1 = colsum(out1) j>=512, row2/3 = same for ql. The window tail
    accumulates NEGATED into rows 1/3 cols 448:512 (same j columns), so
    no extra accumulators or dots are needed.
  - Epilogue per chain: one scalar_tensor_tensor over [4,512] PSUM with
    the [4,512] weight rows (Bv lo/hi, D lo/hi) and per-partition
    accum_out -> 4 partial dot products. Host sums them.
  - Window mask ops (ge/scan/neg_tq/neg_tl) on [128,64] row-layout
    slices, split between DVE and GpSimd.
"""

import os
import threading
from contextlib import ExitStack

import numpy as np

B, L = 16384, 1024
N_CORES = 8
ROWS_PER_CORE = B // N_CORES  # 2048
N_TILES = ROWS_PER_CORE // 128  # 16
ALPHA = 0.65
W = 32  # tail window width

# PSUM accumulation chains (tile counts); small last group shortens tail.
CHAIN_GROUPS = (5, 5, 4, 2)

_compiled = threading.local()


def _reward_rows():
    j = np.arange(L, dtype=np.float64)
    bv = (j + 1.0) / ALPHA
    d = -1.0 / np.log2(j + 2.0) - bv
    return bv.astype(np.float32), d.astype(np.float32)


def _build(rows=ROWS_PER_CORE, num_devices=int(os.environ.get("NUM_DEV", str(N_CORES)))):
    import concourse.tile as tile
    from concourse import bacc, mybir

    f32 = mybir.dt.float32
    f16 = mybir.dt.float16
    u8 = mybir.dt.uint8
    Alu = mybir.AluOpType

    n_tiles = rows // 128
    assert n_tiles == N_TILES and n_tiles % 2 == 0
    n_chains = len(CHAIN_GROUPS)

    nc = bacc.Bacc(
        "TRN2",
        target_bir_lowering=False,
        debug=False,
        enable_asserts=os.environ.get("ASSERTS", "1") == "1",
        num_devices=num_devices,
    )

    def eng(name):
        return {"dve": nc.vector, "gp": nc.gpsimd, "act": nc.scalar}[name]

    lsb_pairs = int(os.environ.get("LSB_PAIRS", "4"))
    n_pairs = n_tiles // 2
    dma_lab_rows = (n_pairs - lsb_pairs) * 256
    u16 = mybir.dt.uint16
    out1_d = nc.dram_tensor("out1", [rows, L], f16, kind="ExternalInput").ap()
    # u16 labels only for the pairs that don't carry labels in the f16 LSB
    lab_d = nc.dram_tensor(
        "lab", [max(dma_lab_rows, 256), L], u16, kind="ExternalInput"
    ).ap()
    # out0 window, host-packed partition-major: [128, n_tiles*W]
    w0_d = nc.dram_tensor("w0", [128, n_tiles * W], f16, kind="ExternalInput").ap()
    # dot weights: rows = [Bv_lo, Bv_hi, D_lo, D_hi]
    wrow_d = nc.dram_tensor("wrow", [4, 512], f32, kind="ExternalInput").ap()
    # output: cols 0:n_tiles = flags, col n_tiles+c (partitions 0:4) = chain dots
    res_d = nc.dram_tensor(
        "res", [128, n_tiles + n_chains], f32, kind="ExternalOutput"
    ).ap()

    with tile.TileContext(nc) as tc, ExitStack() as ctx:
        const = ctx.enter_context(tc.tile_pool(name="const", bufs=1))
        inp = ctx.enter_context(tc.tile_pool(name="inp", bufs=8))
        labp = ctx.enter_context(tc.tile_pool(name="labp", bufs=8))
        labf_p = ctx.enter_context(tc.tile_pool(name="labf", bufs=8))
        work = ctx.enter_context(tc.tile_pool(name="work", bufs=4))
        win = ctx.enter_context(tc.tile_pool(name="win", bufs=4))
        wint = ctx.enter_context(tc.tile_pool(name="wint", bufs=8))
        psum = ctx.enter_context(tc.tile_pool(name="psum", bufs=1, space="PSUM"))

        # ---- first out1 pair goes out before anything else (head latency) ----
        o10_t = inp.tile([128, 2 * L], f16, tag="o1p", name="o10_t")
        nc.sync.dma_start(
            o10_t[:].rearrange("p (g l) -> p g l", g=2),
            out1_d[0:256, :].rearrange("(g p) l -> p g l", p=128),
        )

        # ---- constants ----
        w0_t = const.tile([128, n_tiles * W], f16)
        nc.sync.dma_start(w0_t[:], w0_d[:])
        wrow_t = const.tile([4, 512], f32)
        nc.sync.dma_start(wrow_t[:], wrow_d[:])
        # one-hot stationaries e_c [128,4]: column c all-ones
        e_st = []
        for c in range(4):
            e = const.tile([128, 4], f16, tag=f"e{c}")
            nc.vector.memset(e[:], 0.0)
            nc.vector.memset(e[:, c : c + 1], 1.0)
            e_st.append(e)

        res_t = const.tile([128, n_tiles + n_chains], f32)
        flag_t = res_t[:, 0:n_tiles]

        # PSUM accumulators: one [4,512] tile (one bank) per chain
        ps = [
            psum.tile([4, 512], f32, tag=f"ps{c}", name=f"ps{c}")
            for c in range(n_chains)
        ]

        # chain id per tile
        chain_of = []
        for c, g in enumerate(CHAIN_GROUPS):
            chain_of += [c] * g
        assert len(chain_of) == n_tiles
        chain_start = {}
        chain_end = {}
        for i, c in enumerate(chain_of):
            chain_start.setdefault(c, i)
            chain_end[c] = i

        # deferred window matmuls: per chain, list of (e_idx, moving_ap)
        pending = {c: [] for c in range(n_chains)}

        def flush_chain(c):
            items = pending[c]
            for k, (ei, mov) in enumerate(items):
                nc.tensor.matmul(
                    ps[c][0:4, 512 - W : 512],
                    e_st[ei][:],
                    mov,
                    start=False,
                    stop=k == len(items) - 1,
                )
            pending[c] = None
            junk = work.tile([4, 512], f32, tag=f"junk{c}", name=f"junk{c}")
            nc.vector.scalar_tensor_tensor(
                junk[:],
                ps[c][:],
                1.0,
                wrow_t[:],
                Alu.mult,
                Alu.mult,
                accum_out=res_t[0:4, n_tiles + c : n_tiles + c + 1],
            )

        pair = {}
        for i in range(n_tiles):
            if i % 2 == 0:
                # ---- pair DMA loads on one HWDGE FIFO queue in need-order,
                # so completions arrive in exactly this order ----
                p_idx = i // 2
                lsb = p_idx < lsb_pairs
                r0 = i * 128
                if i == 0:
                    o_t = o10_t
                else:
                    o_t = inp.tile([128, 2 * L], f16, tag="o1p")
                    nc.sync.dma_start(
                        o_t[:].rearrange("p (g l) -> p g l", g=2),
                        out1_d[r0 : r0 + 256, :].rearrange("(g p) l -> p g l", p=128),
                    )
                if lsb:
                    # labels live in the f16 LSB of out1: extract on DVE
                    # (tensor_scalar 4x on the u16 view)
                    l2 = labp.tile([128, 2 * L], u16, tag="labx", name="l2")
                    nc.vector.tensor_scalar(
                        l2[:], o_t[:].bitcast(u16), 1, None, Alu.bitwise_and
                    )
                else:
                    l2 = labp.tile([128, 2 * L], u16, tag="labp", name="l2d")
                    lr0 = (p_idx - lsb_pairs) * 256
                    nc.sync.dma_start(
                        l2[:].rearrange("p (g l) -> p g l", g=2),
                        lab_d[lr0 : lr0 + 256, :].rearrange(
                            "(g p) l -> p g l", p=128
                        ),
                    )
                # fused pair ops on DVE: one op over both tiles
                ql2 = work.tile([128, 2 * L], f16, tag="ql2", name="ql2")
                nc.vector.tensor_tensor(ql2[:], o_t[:], l2[:], Alu.mult)
                o_v = o_t[:].rearrange("p (g l) -> p g l", g=2)
                ge2 = win.tile([128, 2 * W], f16, tag="ge2", name="ge2")
                nc.vector.tensor_tensor(
                    ge2[:].rearrange("p (g w) -> p g w", g=2),
                    w0_t[:, i * W : (i + 2) * W].rearrange(
                        "p (g w) -> p g w", g=2
                    ),
                    o_v[:, :, L - W : L],
                    Alu.is_ge,
                )
                s2 = win.tile([128, 2 * W], f16, tag="spair")
                ntq2 = wint.tile([128, 2 * W], f16, tag="ntq2", name="ntq2")
                ntl2 = wint.tile([128, 2 * W], f16, tag="ntl2", name="ntl2")
                pair = {
                    "o": o_t,
                    "l": l2,
                    "ql": ql2,
                    "s": s2,
                    "ge": ge2,
                    "ntq": ntq2,
                    "ntl": ntl2,
                }

            half = i % 2
            c = chain_of[i]
            out1_t = pair["o"][:, half * L : (half + 1) * L]
            ql = pair["ql"][:, half * L : (half + 1) * L]

            # ---- window pipeline (scan + tail products per tile) ----
            out1_w = out1_t[:, L - W : L]
            ge_w = pair["ge"][:, half * W : (half + 1) * W]
            s_w = pair["s"][:, half * W : (half + 1) * W]
            nc.vector.tensor_tensor_scan(
                s_w[:, ::-1], ge_w[:, ::-1], ge_w[:, ::-1], 0.0, Alu.max, Alu.max
            )
            # neg_tq = (s - s[0]) * out1_w  ( = -(strict tail mask) * out1_w;
            # s[0] = 1 - allones_flag, so suspicious rows contribute 0 )
            ntq = pair["ntq"][:, half * W : (half + 1) * W]
            nc.vector.scalar_tensor_tensor(
                ntq,
                s_w,
                s_w[:, 0:1],
                out1_w,
                Alu.subtract,
                Alu.mult,
            )
            if half == 1:
                # flags for the pair: flag = (s[0] == 0), 1 iff no
                # zero-decision in the window (suspicious or all-ones row)
                nc.vector.tensor_scalar(
                    flag_t[:, i - 1 : i + 1],
                    pair["s"][:, 0 : 2 * W : W],
                    0.0,
                    None,
                    Alu.is_equal,
                )
                # fused tail-label product for the pair
                l_v = pair["l"][:].rearrange("p (g l) -> p g l", g=2)
                nc.vector.tensor_tensor(
                    pair["ntl"][:].rearrange("p (g w) -> p g w", g=2),
                    pair["ntq"][:].rearrange("p (g w) -> p g w", g=2),
                    l_v[:, :, L - W : L],
                    Alu.mult,
                )
            ntl = pair["ntl"][:, half * W : (half + 1) * W]

            # ---- main matmuls into this chain's accumulator ----
            pst = ps[c]
            st = i == chain_start[c]
            nc.tensor.matmul(pst[:], e_st[0][:], out1_t[:, 0:512], start=st, stop=False)
            nc.tensor.matmul(pst[:], e_st[1][:], out1_t[:, 512:L], start=False, stop=False)
            nc.tensor.matmul(pst[:], e_st[2][:], ql[:, 0:512], start=False, stop=False)
            nc.tensor.matmul(pst[:], e_st[3][:], ql[:, 512:L], start=False, stop=False)
            # window tails (negated, cols 512-W:512) are deferred two tiles so
            # the PE never waits on the window scan chain
            pending[c].append((1, ntq))
            pending[c].append((3, ntl))
            if i - 2 >= 0 and chain_end[chain_of[i - 2]] == i - 2:
                flush_chain(chain_of[i - 2])

        for c in range(n_chains):
            if pending[c] is not None:
                flush_chain(c)

        nc.sync.dma_start(res_d[:], res_t[:])

    nc.compile()
    return nc


def _get_nc():
    if getattr(_compiled, "nc", None) is None:
        _compiled.nc = _build()
    return _compiled.nc


def _in_maps(output, labels):
    import os as _os

    lsb_pairs = int(_os.environ.get("LSB_PAIRS", "4"))
    lsb_rows = lsb_pairs * 256
    out1 = np.ascontiguousarray(output[:, :, 1]).astype(np.float16)
    lab = labels.astype(np.uint16)
    out0w = np.ascontiguousarray(output[:, L - W :, 0]).astype(np.float16)
    bv, dd = _reward_rows()
    wrow = np.stack([bv[0:512], bv[512:L], dd[0:512], dd[512:L]]).astype(np.float32)
    rp = ROWS_PER_CORE
    maps = []
    for c in range(N_CORES):
        o1c = out1[c * rp : (c + 1) * rp].copy()
        labc = lab[c * rp : (c + 1) * rp]
        if lsb_rows:
            # embed labels in the f16 mantissa LSB of out1 for the LSB pairs
            v = o1c[:lsb_rows].view(np.uint16)
            v &= np.uint16(0xFFFE)
            v |= labc[:lsb_rows]
        lab_dma = labc[lsb_rows:]
        if lab_dma.shape[0] == 0:
            lab_dma = np.zeros((256, L), dtype=np.uint16)
        # pack out0 window partition-major: [128, n_tiles*W]
        w0c = (
            out0w[c * rp : (c + 1) * rp]
            .reshape(N_TILES, 128, W)
            .transpose(1, 0, 2)
            .reshape(128, N_TILES * W)
        )
        maps.append(
            {
                "out1": o1c,
                "lab": np.ascontiguousarray(lab_dma),
                "w0": np.ascontiguousarray(w0c),
                "wrow": wrow,
            }
        )
    return maps


def _host_fallback(output, labels):
    temp = output[:, :, 1] > output[:, :, 0]
    allones = temp.all(axis=1)
    z = ~temp
    last_zero = (L - 1) - np.argmax(z[:, ::-1], axis=1)
    idx = np.where(allones, L, last_zero)
    mask = np.arange(L)[None, :] <= idx[:, None]
    j = np.arange(L, dtype=np.float64)
    r1 = np.where(labels == 1, -1.0 / np.log2(j + 2.0), (j + 1.0) / ALPHA)
    return np.float32((output[:, :, 1].astype(np.float64) * mask * r1).sum() / B)


def _combine(results, output, labels):
    total = 0.0
    suspicious = 0
    n_chains = len(CHAIN_GROUPS)
    for c, r in enumerate(results):
        res = np.asarray(r["res"], dtype=np.float64)
        total += res[0:4, N_TILES : N_TILES + n_chains].sum()
        flags = res[:, 0:N_TILES]
        if flags.max() > 0:
            rp = ROWS_PER_CORE
            o = output[c * rp : (c + 1) * rp]
            allones_rows = (o[:, :, 1] > o[:, :, 0]).all(axis=1)
            flagged = flags.T.reshape(-1) > 0  # row-major within this core
            suspicious += int((flagged & ~allones_rows).sum())
    if suspicious > 0:
        return _host_fallback(output, labels)
    return np.float32(total / B)


def kernel(output: np.ndarray, labels: np.ndarray) -> np.ndarray:
    from concourse.bass_utils import run_bass_kernel_spmd

    assert output.shape == (B, L, 2), output.shape
    nc = _get_nc()
    res = run_bass_kernel_spmd(
        nc, _in_maps(output, labels), core_ids=list(range(N_CORES))
    )
    return _combine(res.results, output, labels)


# revision 13
# speedup vs baseline: 3.5256x; 3.5256x over previous
"""BiCutLoss Trainium2 kernel (8-core data parallel over batch).

Reference semantics (B=16384, L=1024):
    temp[b,j]  = argmax(output[b,j,:])          # 1 iff out1 > out0 (ties -> 0)
    idx[b]     = L if row all-ones else index of last zero
    mask[b,j]  = j <= idx[b]
    r1[b,j]    = -1/log2(j+2)  if labels==1 else (j+1)/alpha
    loss       = sum(output[...,1] * mask * r1) / B

Restructuring: masked_sum = full_sum - tail_sum; the tail (j > idx) is
confined to the last W columns whenever each row has a zero decision in
its last W positions (P(violation) ~= 2^-W per random row; per-row flags
catch violations and the host falls back to an exact evaluation, so the
kernel is correct for all inputs).

Design (memory-regime):
  - out1 as f16 (4MB/core); labels EMBEDDED in the f16 mantissa LSB by
    the host (free) and extracted on-device with one 4x-mode
    tensor_scalar bitwise_and per group -> no label DMA at all.
    LSB dither perturbs out1 by <=2^-11 relative (~0.05% on the loss,
    tolerance is 2e-2).
  - ql = out1 * lab on DVE at 2x mode (both operands 2-byte).
  - Column sums via PE matmuls with one-hot [128,8] stationaries into a
    single PSUM tile [8,512] per chain: rows 0/1 = colsum(out1) lo/hi,
    rows 2/3 = colsum(ql) lo/hi, row 4 = negated window tail of out1
    (group halves side by side in cols 0:gsz*W), row 6 = same for ql.
  - Epilogue per chain: one scalar_tensor_tensor over [8,512] PSUM with
    an [8,512] weight matrix (Bv lo/hi, D lo/hi, duplicated window
    weights) and per-partition accum_out -> 8 partial dots. Host sums.
  - Window mask: ge/scan/neg-tail on [128,W] row-layout slices (DVE).
    Window matmuls are deferred two tiles so the PE never waits on the
    scan chain.
  - All input DMAs on one HWDGE FIFO queue in need-order; first two
    tile-groups are single tiles to cut head latency.
"""

import os
import threading
from contextlib import ExitStack

import numpy as np

B, L = 16384, 1024
N_CORES = 8
ROWS_PER_CORE = B // N_CORES  # 2048
N_TILES = ROWS_PER_CORE // 128  # 16
ALPHA = 0.65
W = 16  # tail window width

# tile groups: (start, size); singles first for head latency
GROUPS = [(0, 1), (1, 1), (2, 2), (4, 2), (6, 2), (8, 2), (10, 2), (12, 2), (14, 2)]
# PSUM accumulation chains: (first tile, last tile)
CHAINS = [(0, 7), (8, 15)]

_compiled = threading.local()


def _reward_rows():
    j = np.arange(L, dtype=np.float64)
    bv = (j + 1.0) / ALPHA
    d = -1.0 / np.log2(j + 2.0) - bv
    return bv, d


def _build(rows=ROWS_PER_CORE, num_devices=N_CORES):
    import concourse.tile as tile
    from concourse import bacc, mybir

    f32 = mybir.dt.float32
    f16 = mybir.dt.float16
    u16 = mybir.dt.uint16
    Alu = mybir.AluOpType

    n_tiles = rows // 128
    assert n_tiles == N_TILES
    n_chains = len(CHAINS)

    nc = bacc.Bacc(
        "TRN2",
        target_bir_lowering=False,
        debug=False,
        enable_asserts=False,
        num_devices=num_devices,
    )

    out1_d = nc.dram_tensor("out1", [rows, L], f16, kind="ExternalInput").ap()
    # out0 window, host-packed partition-major: [128, n_tiles*W]
    w0_d = nc.dram_tensor("w0", [128, n_tiles * W], f16, kind="ExternalInput").ap()
    # dot weights [8,512]: rows Bv_lo, Bv_hi, D_lo, D_hi, wBv-dup, 0, wD-dup, 0
    wrow_d = nc.dram_tensor("wrow", [8, 512], f32, kind="ExternalInput").ap()
    # output: cols 0:n_tiles = flags, col n_tiles+c (partitions 0:8) = chain dots
    res_d = nc.dram_tensor(
        "res", [128, n_tiles + n_chains], f32, kind="ExternalOutput"
    ).ap()

    chain_of = {}
    chain_start = {}
    chain_end = {}
    for c, (a, b) in enumerate(CHAINS):
        chain_start[c] = a
        chain_end[c] = b
        for i in range(a, b + 1):
            chain_of[i] = c

    with tile.TileContext(nc) as tc, ExitStack() as ctx:
        const = ctx.enter_context(tc.tile_pool(name="const", bufs=1))
        inp = ctx.enter_context(tc.tile_pool(name="inp", bufs=8))
        labx_p = ctx.enter_context(tc.tile_pool(name="labx", bufs=4))
        work = ctx.enter_context(tc.tile_pool(name="work", bufs=4))
        win = ctx.enter_context(tc.tile_pool(name="win", bufs=4))
        wint = ctx.enter_context(tc.tile_pool(name="wint", bufs=6))
        psum = ctx.enter_context(tc.tile_pool(name="psum", bufs=1, space="PSUM"))

        # ---- first two single-tile loads go out before everything else ----
        head_tiles = {}
        for g0, gsz in GROUPS[:2]:
            o_t = inp.tile([128, L], f16, tag="o1s", name="o_head")
            nc.sync.dma_start(o_t[:], out1_d[g0 * 128 : (g0 + 1) * 128, :])
            head_tiles[g0] = o_t

        # ---- constants ----
        w0_t = const.tile([128, n_tiles * W], f16)
        nc.sync.dma_start(w0_t[:], w0_d[:])
        wrow_t = const.tile([8, 512], f32)
        nc.sync.dma_start(wrow_t[:], wrow_d[:])
        # one-hot stationaries [128,8]: column c all-ones
        e_st = []
        for c in (0, 1, 2, 3, 4, 6):
            e = const.tile([128, 8], f16, tag=f"e{c}", name=f"e{c}")
            nc.vector.memset(e[:], 0.0)
            nc.vector.memset(e[:, c : c + 1], 1.0)
            e_st.append(e)
        e_idx = {0: 0, 1: 1, 2: 2, 3: 3, 4: 4, 6: 5}

        res_t = const.tile([128, n_tiles + n_chains], f32)
        flag_t = res_t[:, 0:n_tiles]

        # PSUM accumulators: one [8,512] tile (one bank) per chain
        ps = [
            psum.tile([8, 512], f32, tag=f"ps{c}", name=f"ps{c}")
            for c in range(n_chains)
        ]

        # deferred window matmuls: per chain, list of (one-hot col, moving)
        pending = {c: [] for c in range(n_chains)}
        flushed = set()

        def flush_chain(c):
            items = pending[c]
            for k, (ei, mov) in enumerate(items):
                nwc = mov.shape[-1]
                nc.tensor.matmul(
                    ps[c][0:8, 0:nwc],
                    e_st[e_idx[ei]][:],
                    mov,
                    start=False,
                    stop=k == len(items) - 1,
                )
            pending[c] = []
            flushed.add(c)
            junk = work.tile([8, 512], f32, tag=f"junk{c}", name=f"junk{c}")
            nc.vector.scalar_tensor_tensor(
                junk[:],
                ps[c][:],
                1.0,
                wrow_t[:],
                Alu.mult,
                Alu.mult,
                accum_out=res_t[0:8, n_tiles + c : n_tiles + c + 1],
            )

        for g0, gsz in GROUPS:
            gl = gsz * L
            # ---- group DMA load (singles are preloaded) ----
            if g0 in head_tiles:
                o_t = head_tiles[g0]
            else:
                o_t = inp.tile([128, gl], f16, tag=f"o1g{gsz}", name="o_t")
                nc.sync.dma_start(
                    o_t[:].rearrange("p (g l) -> p g l", g=gsz),
                    out1_d[g0 * 128 : (g0 + gsz) * 128, :].rearrange(
                        "(g p) l -> p g l", p=128
                    ),
                )

            # ---- label extraction from the f16 LSB (4x tensor_scalar) ----
            lx = labx_p.tile([128, gl], u16, tag="labx", name="lx")
            nc.vector.tensor_scalar(
                lx[:], o_t[:].bitcast(u16), 1, None, Alu.bitwise_and
            )
            # ---- ql = out1 * lab (2x tensor_tensor; u16 converts by value)
            ql_g = work.tile([128, gl], f16, tag="ql", name="ql_g")
            nc.vector.tensor_tensor(ql_g[:], o_t[:], lx[:], Alu.mult)

            # ---- window pipeline ----
            o_v = o_t[:].rearrange("p (g l) -> p g l", g=gsz)
            ge_g = win.tile([128, gsz * W], f16, tag="ge", name="ge_g")
            nc.vector.tensor_tensor(
                ge_g[:].rearrange("p (g w) -> p g w", g=gsz),
                w0_t[:, g0 * W : (g0 + gsz) * W].rearrange(
                    "p (g w) -> p g w", g=gsz
                ),
                o_v[:, :, L - W : L],
                Alu.is_ge,
            )
            s_g = win.tile([128, gsz * W], f16, tag="sg", name="s_g")
            ntq_g = wint.tile([128, gsz * W], f16, tag="ntq", name="ntq_g")
            for k in range(gsz):
                ge_w = ge_g[:, k * W : (k + 1) * W]
                s_w = s_g[:, k * W : (k + 1) * W]
                nc.vector.tensor_tensor_scan(
                    s_w[:, ::-1], ge_w[:, ::-1], ge_w[:, ::-1], 0.0, Alu.max, Alu.max
                )
                # neg_tq = (s - s[0]) * out1_w: -(strict tail mask) * out1_w;
                # s[0] = 1 - allones_flag, so suspicious rows contribute 0
                nc.vector.scalar_tensor_tensor(
                    ntq_g[:, k * W : (k + 1) * W],
                    s_w,
                    s_w[:, 0:1],
                    o_t[:, k * L + L - W : (k + 1) * L],
                    Alu.subtract,
                    Alu.mult,
                )
            # flags: flag = (s[0] == 0), 1 iff no zero-decision in window
            nc.vector.tensor_scalar(
                flag_t[:, g0 : g0 + gsz],
                s_g[:, 0 : gsz * W : W],
                0.0,
                None,
                Alu.is_equal,
            )
            # fused tail-label product for the group
            ntl_g = wint.tile([128, gsz * W], f16, tag="ntl", name="ntl_g")
            lx_v = lx[:].rearrange("p (g l) -> p g l", g=gsz)
            nc.vector.tensor_tensor(
                ntl_g[:].rearrange("p (g w) -> p g w", g=gsz),
                ntq_g[:].rearrange("p (g w) -> p g w", g=gsz),
                lx_v[:, :, L - W : L],
                Alu.mult,
            )

            # ---- main matmuls per tile of the group ----
            for k in range(gsz):
                i = g0 + k
                c = chain_of[i]
                pst = ps[c]
                st = i == chain_start[c]
                out1_t = o_t[:, k * L : (k + 1) * L]
                ql = ql_g[:, k * L : (k + 1) * L]
                nc.tensor.matmul(
                    pst[:], e_st[0][:], out1_t[:, 0:512], start=st, stop=False
                )
                nc.tensor.matmul(
                    pst[:], e_st[1][:], out1_t[:, 512:L], start=False, stop=False
                )
                nc.tensor.matmul(
                    pst[:], e_st[2][:], ql[:, 0:512], start=False, stop=False
                )
                nc.tensor.matmul(
                    pst[:], e_st[3][:], ql[:, 512:L], start=False, stop=False
                )
            # window tails (negated) deferred; group halves side by side land
            # in psum cols 0:gsz*W of rows 4 (out1) / 6 (ql), whose dot
            # weights are the duplicated window Bv/D
            c = chain_of[g0]
            pending[c].append((4, ntq_g[:]))
            pending[c].append((6, ntl_g[:]))
            # flush a chain two tiles after it ended
            for cc in range(n_chains):
                if cc not in flushed and chain_end[cc] + 2 <= g0:
                    flush_chain(cc)

        for cc in range(n_chains):
            if cc not in flushed:
                flush_chain(cc)

        nc.sync.dma_start(res_d[:], res_t[:])

    nc.compile()
    return nc


def _get_nc():
    if getattr(_compiled, "nc", None) is None:
        _compiled.nc = _build()
    return _compiled.nc


def _in_maps(output, labels):
    out1 = np.ascontiguousarray(output[:, :, 1]).astype(np.float16)
    lab = labels.astype(np.uint16)
    # embed labels in the f16 mantissa LSB of out1
    v = out1.view(np.uint16)
    v &= np.uint16(0xFFFE)
    v |= lab
    out0w = np.ascontiguousarray(output[:, L - W :, 0]).astype(np.float16)
    bv, dd = _reward_rows()
    wrow = np.zeros((8, 512), dtype=np.float64)
    wrow[0] = bv[0:512]
    wrow[1] = bv[512:L]
    wrow[2] = dd[0:512]
    wrow[3] = dd[512:L]
    wrow[4, 0 : 2 * W] = np.concatenate([bv[L - W :], bv[L - W :]])
    wrow[6, 0 : 2 * W] = np.concatenate([dd[L - W :], dd[L - W :]])
    wrow = wrow.astype(np.float32)
    rp = ROWS_PER_CORE
    maps = []
    for c in range(N_CORES):
        w0c = (
            out0w[c * rp : (c + 1) * rp]
            .reshape(N_TILES, 128, W)
            .transpose(1, 0, 2)
            .reshape(128, N_TILES * W)
        )
        maps.append(
            {
                "out1": out1[c * rp : (c + 1) * rp],
                "w0": np.ascontiguousarray(w0c),
                "wrow": wrow,
            }
        )
    return maps


def _host_fallback(output, labels):
    temp = output[:, :, 1] > output[:, :, 0]
    allones = temp.all(axis=1)
    z = ~temp
    last_zero = (L - 1) - np.argmax(z[:, ::-1], axis=1)
    idx = np.where(allones, L, last_zero)
    mask = np.arange(L)[None, :] <= idx[:, None]
    j = np.arange(L, dtype=np.float64)
    r1 = np.where(labels == 1, -1.0 / np.log2(j + 2.0), (j + 1.0) / ALPHA)
    return np.float32((output[:, :, 1].astype(np.float64) * mask * r1).sum() / B)


def _combine(results, output, labels):
    total = 0.0
    suspicious = 0
    n_chains = len(CHAINS)
    for c, r in enumerate(results):
        res = np.asarray(r["res"], dtype=np.float64)
        total += res[0:8, N_TILES : N_TILES + n_chains].sum()
        flags = res[:, 0:N_TILES]
        if flags.max() > 0:
            rp = ROWS_PER_CORE
            o = output[c * rp : (c + 1) * rp]
            allones_rows = (o[:, :, 1] > o[:, :, 0]).all(axis=1)
            flagged = flags.T.reshape(-1) > 0  # row-major within this core
            suspicious += int((flagged & ~allones_rows).sum())
    if suspicious > 0:
        return _host_fallback(output, labels)
    return np.float32(total / B)


def kernel(output: np.ndarray, labels: np.ndarray) -> np.ndarray:
    from concourse.bass_utils import run_bass_kernel_spmd

    assert output.shape == (B, L, 2), output.shape
    nc = _get_nc()
    res = run_bass_kernel_spmd(
        nc, _in_maps(output, labels), core_ids=list(range(N_CORES))
    )
    return _combine(res.results, output, labels)
